# revision 10
# baseline (speedup 1.0000x reference)
"""Trainium2 Bass kernel: bidirectional conv-BN-relu message passing over H.

Reference semantics (per batch item, channels C, scan over H):
  forward:  new[0] = x[0];   new[h] = relu(bn(conv(new[h-1]))) + x[h]
  backward: out[H-1] = new[H-1]; out[h] = relu(bn(conv(out[h+1]))) + new[h]
conv = 1D conv along W, kernel 9, pad 4, C->C channels; BN (eval mode)
is a per-channel affine y*s + t.

Strategy: data-parallel over B across 8 cores (2 batch items per core).
Each conv step = 9 shifted-window matmuls accumulated in PSUM
(lhsT = per-tap [I,O] weights, rhs = state slice windows).

Matmul operands are bf16: fp32 weights disable the PE's fast-weight-load
path, while bf16 keeps the conv recurrence within ~7e-3 relative error
because PSUM accumulation and the x-carry adds stay fp32.

The affine+relu+carry tail is algebraically folded into ONE DVE op:
 - the BN scale s is folded into the weights host-side (W' = s[o]*W),
 - the state is stored shifted: n = new - r, where r solves
   r = t + M r (M[o,i] = sum_k W'[o,i,k]); then zero-padding of `new`
   corresponds to constant pads -r on n, and the update collapses to
     n_h = max(psum, -r) + carry          (single scalar_tensor_tensor)
   r is added back to the gathered output on the host.

Pad columns are NEVER streamed through the PE: each tap's matmul is
trimmed to the columns whose input window stays inside the valid
interior (N = 256-|k-4|), saving 20 of 2304 streamed columns per step.
The missing pad contributions are a precomputed [C,4]-per-edge constant
(host-solved from W' and r) added into PSUM by two small DVE ops before
the state-update STT — exact, not an approximation. This also deletes
all pad-fill maintenance of the state tiles.

The backward carry new_h = n_h + r is staged per step on the (otherwise
idle) Activation engine so the tensor engine runs nothing but the 9-tap
conv groups.

Startup/drain engineering (trace-driven): the engines cannot issue
anything before ~7.6us (framework preamble) and the first DMA packet
hits ~1.45us after issue, so the startup is bandwidth-bound on the
critical bytes. Those are minimized: x carries ship as bf16 (validated
+2e-4 error), h=0 state ships interior-only, and the three transfers
spread over three queues (sync: state+x, scalar: taps 0-3, gpsimd:
params+taps 4-8). A burst of dummy matmuls (on a vector-memset tile)
keeps the PE busy from ~7.6us so the HAM clock-gate lifts (1.2 ->
2.4 GHz) as early as possible. Input slices arrive in growing batches;
outputs leave pad-inclusive (contiguous 528B*OB runs) on two queues
mid-stream, and the final h=0 slices fan out across four queues so the
drain rides four DMA rings in parallel.
"""

import os
from contextlib import ExitStack

import numpy as np
import ml_dtypes

import bass_rust
import concourse.bass as bass
import concourse.tile as tile
from concourse import mybir
from concourse.bass_utils import run_bass_kernel_spmd

B, C, H, W = 16, 128, 64, 256
K, PAD = 9, 4
NCORES = 8
BPC = B // NCORES  # batch items per core
WP = W + 2 * PAD
EPS = 1e-5
OB = 4  # output-slice DMA batch
NWARM = 12  # dummy matmuls bridging the PE from preamble-end to the
# arrival of the first weight/state DMAs (~2.4us at the cold 213ns/mm)

F32 = mybir.dt.float32
BF16 = mybir.dt.bfloat16
NP_BF16 = ml_dtypes.bfloat16
IDENT = mybir.ActivationFunctionType.Identity

_NC_CACHE: dict = {}
LAST_RESULTS = None  # stashed BassKernelResults for test.py introspection


def _xbounds(h_dim):
    """Input-batch spans for h>=1: small leading batches so the first conv
    steps aren't gated on a bulk transfer, then steady groups of 4."""
    bounds, sizes, lo = [], [1, 2, 4], 1
    while lo < h_dim:
        sz = sizes[0] if sizes else 4
        if sizes:
            sizes = sizes[1:]
        bounds.append((lo, min(lo + sz, h_dim)))
        lo += sz
    return bounds


# Per-tap trimmed geometry: tap k covers psum cols [PO[k], PO[k]+NK[k])
# reading state interior cols [RO[k], RO[k]+NK[k]) of the padded row.
RO = [max(k, PAD) for k in range(K)]
PO = [max(0, PAD - k) for k in range(K)]
NK = [min(k + W, W + PAD) - max(k, PAD) for k in range(K)]


def _build_nc(bpc=BPC, h_dim=H, w_dim=W):
    wp = w_dim + 2 * PAD
    nc = bass.Bass()
    x_d = nc.dram_tensor("x", [bpc, C, h_dim - 1, w_dim], BF16, kind="ExternalInput")
    n0_d = nc.dram_tensor("n0", [bpc, C, w_dim], BF16, kind="ExternalInput")
    w_d = nc.dram_tensor("w", [C, K, C], BF16, kind="ExternalInput")
    # pr: [-r, +r, cL(4), cR(4)] per channel
    pr_d = nc.dram_tensor("pr", [C, 10], F32, kind="ExternalInput")
    o_d = nc.dram_tensor("o", [bpc, C, h_dim, wp], BF16, kind="ExternalOutput")

    add = mybir.AluOpType.add
    mx = mybir.AluOpType.max

    xb_list = _xbounds(h_dim)
    xb_of_h = {}
    for i, (lo, hi) in enumerate(xb_list):
        for h in range(lo, hi):
            xb_of_h[h] = (i, lo, hi)

    with ExitStack() as ctx:
        tc = ctx.enter_context(tile.TileContext(nc))
        singles = ctx.enter_context(tc.tile_pool(name="singles", bufs=1))
        big = ctx.enter_context(tc.tile_pool(name="big", bufs=1))
        xs_pool = ctx.enter_context(tc.tile_pool(name="xs", bufs=6))
        nr_pool = ctx.enter_context(tc.tile_pool(name="nr", bufs=4))
        pp = ctx.enter_context(tc.tile_pool(name="pp", bufs=8, space="PSUM"))

        # --- DMA ordering: three queues in parallel. sync: h=0 state
        # (gates the first conv group) then the x-carry batches; scalar:
        # weight taps 0-3 (gate the first matmuls); gpsimd: the tiny param
        # vector then taps 4-8 (needed ~1us after tap 0).
        new = []
        for c in range(bpc):
            nt = big.tile([C, h_dim, wp], BF16, tag=f"new{c}", name=f"new{c}")
            nc.sync.dma_start(out=nt[:, 0, PAD : PAD + w_dim], in_=n0_d[c])
            new.append(nt)

        prt = singles.tile([C, 10], F32, tag="prt", name="prt")
        nc.gpsimd.dma_start(out=prt, in_=pr_d[:, :])
        bt = prt[:, 0:1]
        rt = prt[:, 1:2]
        cl = prt[:, 2:6]
        cr = prt[:, 6:10]

        wt = singles.tile([C, K, C], BF16, tag="wt", name="wt")
        nc.scalar.dma_start(out=wt[:, 0:4, :], in_=w_d[:, 0:4, :])
        nc.gpsimd.dma_start(out=wt[:, 4:K, :], in_=w_d[:, 4:K, :])
        wr = [wt[:, k, :] for k in range(K)]

        # --- HAM warmup: dummy matmuls on a zeroed tile keep the PE busy
        # while the DMAs land, releasing the activity clock-gate.
        dummy = singles.tile([C, w_dim], BF16, tag="dummy", name="dummy")
        nc.vector.memset(dummy, 0.0)
        wm = pp.tile([C, w_dim], F32, tag="pt", name="wm", bufs=6)
        for _ in range(NWARM):
            nc.tensor.matmul(wm, dummy[:, 0:C], dummy, start=True, stop=True)

        # Both 4-wide edge windows of a psum tile (and of the [C,8] constant)
        # as one 3D AP, so the pad fixup is a single DVE op — the DVE chain
        # per group (fix + STT) must stay comfortably under the ~975ns PE
        # group time or every group boundary stalls on the state-row RAW.
        def edges2(ap4, gap):
            return bass.AP(ap4.tensor, ap4.offset, [ap4.ap[0], [gap, 2], [1, PAD]])

        clr = edges2(cl, PAD)  # prt cols 2:6 + 6:10

        def conv_group(src_row, pt):
            """9 edge-trimmed taps accumulated in PSUM + pad-constant fixup."""
            for k in range(K):
                nc.tensor.matmul(
                    pt[:, PO[k] : PO[k] + NK[k]],
                    wr[k],
                    src_row[:, RO[k] : RO[k] + NK[k]],
                    start=(k == 0),
                    stop=(k == K - 1),
                )
            pe = edges2(pt[:, 0:PAD], w_dim - PAD)
            nc.vector.tensor_add(out=pe, in0=pe, in1=clr)

        # Forward scan over H (both chains interleaved per h).
        xtiles: list[dict[int, object]] = [dict() for _ in range(bpc)]
        for h in range(1, h_dim):
            bi, lo, hi = xb_of_h[h]
            if h == lo:
                for c in range(bpc):
                    xb = xs_pool.tile([C, 4, w_dim], BF16, tag="xb", name="xb")
                    nc.sync.dma_start(
                        out=xb[:, 0 : hi - lo, :], in_=x_d[c][:, lo - 1 : hi - 1, :]
                    )
                    xtiles[c][bi] = xb
            for c in range(bpc):
                pt = pp.tile([C, w_dim], F32, tag="pt", name="pt", bufs=6)
                conv_group(new[c][:, h - 1, :], pt)
                nc.vector.scalar_tensor_tensor(
                    out=new[c][:, h, PAD : PAD + w_dim],
                    in0=pt,
                    scalar=bt,
                    in1=xtiles[c][bi][:, h - lo, :],
                    op0=mx,
                    op1=add,
                )

        # Backward scan; out[h] overwrites new[h] in place, then streams out
        # pad-inclusive in batches of OB slices (contiguous runs DMA much
        # faster than pad-strided 512B packets), one queue per chain.
        oq = [nc.scalar, nc.sync]
        for h in range(h_dim - 2, 0, -1):
            for c in range(bpc):
                # Stage the true backward carry new_h = n_h + r on the ACT
                # engine (reads the forward state before it's overwritten).
                nr = nr_pool.tile([C, w_dim], F32, tag="nr", name="nr")
                nc.scalar.activation(
                    out=nr, in_=new[c][:, h, PAD : PAD + w_dim],
                    func=IDENT, bias=rt, scale=1.0,
                )
                pt = pp.tile([C, w_dim], F32, tag="pt", name="pt", bufs=6)
                conv_group(new[c][:, h + 1, :], pt)
                nc.vector.scalar_tensor_tensor(
                    out=new[c][:, h, PAD : PAD + w_dim],
                    in0=pt,
                    scalar=bt,
                    in1=nr,
                    op0=mx,
                    op1=add,
                )
            if h == 2:
                # Split the final OB-batch so the very last transfer (which
                # gates the drain) is only 2 slices.
                for c in range(bpc):
                    oq[c % 2].dma_start(
                        out=o_d[c][:, 2:4, :], in_=new[c][:, 2:4, :]
                    )
            elif h == 1:
                for c in range(bpc):
                    oq[c % 2].dma_start(
                        out=o_d[c][:, 1:2, :], in_=new[c][:, 1:2, :]
                    )
            elif h % OB == 0:
                hi = min(h + OB, h_dim)
                for c in range(bpc):
                    oq[c % 2].dma_start(
                        out=o_d[c][:, h:hi, :], in_=new[c][:, h:hi, :]
                    )

        # Final step (h=0) in two half-width PSUM groups per chain so the
        # very last DVE op and output transfer are half-sized — they sit on
        # the kernel's drain critical path. Each of the four half-slices
        # leaves on its own DMA queue so the drain transfers run on four
        # rings in parallel.
        hw2 = w_dim // 2
        # Only sync/scalar/gpsimd can issue DMAs. The two half1 pieces are
        # the trailing ones — keep them on different queues.
        oq0 = [[nc.scalar, nc.gpsimd], [nc.sync, nc.scalar]]
        for c in range(bpc):
            nr = nr_pool.tile([C, w_dim], F32, tag="nr", name="nr")
            nc.scalar.activation(
                out=nr, in_=new[c][:, 0, PAD : PAD + w_dim],
                func=IDENT, bias=rt, scale=1.0,
            )
            for half in range(2):
                lo = half * hw2
                pt = pp.tile([C, hw2], F32, tag="pth", name="pth", bufs=2)
                for k in range(K):
                    # Trim within this half: the left edge only exists in
                    # half 0, the right edge only in half 1.
                    if half == 0:
                        p0, p1 = PO[k], hw2
                        r0 = RO[k]
                    else:
                        p0, p1 = 0, min(hw2, hw2 + PAD - k)
                        r0 = k + hw2
                    nc.tensor.matmul(
                        pt[:, p0:p1],
                        wr[k],
                        new[c][:, 1, r0 : r0 + (p1 - p0)],
                        start=(k == 0),
                        stop=(k == K - 1),
                    )
                if half == 0:
                    nc.vector.tensor_add(
                        out=pt[:, 0:PAD], in0=pt[:, 0:PAD], in1=cl
                    )
                else:
                    nc.vector.tensor_add(
                        out=pt[:, hw2 - PAD : hw2],
                        in0=pt[:, hw2 - PAD : hw2],
                        in1=cr,
                    )
                nc.vector.scalar_tensor_tensor(
                    out=new[c][:, 0, PAD + lo : PAD + lo + hw2],
                    in0=pt,
                    scalar=bt,
                    in1=nr[:, lo : lo + hw2],
                    op0=mx,
                    op1=add,
                )
                # Pad-inclusive half-slice: left half carries the left pads,
                # right half the right pads.
                if half == 0:
                    oq0[c][half].dma_start(
                        out=o_d[c][:, 0, 0 : PAD + hw2],
                        in_=new[c][:, 0, 0 : PAD + hw2],
                    )
                else:
                    oq0[c][half].dma_start(
                        out=o_d[c][:, 0, PAD + hw2 : wp],
                        in_=new[c][:, 0, PAD + hw2 : wp],
                    )

    # TRN2 caps most instructions at one semaphore wait (matmuls lower to an
    # LDWEIGHTS struct with a single wait slot); split any excess onto
    # EventSemaphore instructions like bacc does.
    bass_rust.generate_event_semaphores(nc)
    return nc


def _get_nc():
    key = (BPC, H, W)
    if key not in _NC_CACHE:
        _NC_CACHE[key] = _build_nc()
    return _NC_CACHE[key]


def _prep_params(conv_w, gamma, beta, run_mean, run_var):
    """Fold BN scale into the weights, solve the state shift r, and build
    the pad-contribution edge constants.

    Returns (w_t [I,K,O] bf16 with s folded, pr [C,10] f32, r [C] f64).
    """
    s = gamma.astype(np.float64) / np.sqrt(run_var.astype(np.float64) + EPS)
    t = beta.astype(np.float64) - run_mean.astype(np.float64) * s
    w_s = s[:, None, None] * conv_w.astype(np.float64)  # [O,I,K]
    m = w_s.sum(axis=2)  # [O,I]
    r = np.linalg.solve(np.eye(C) - m, t)
    w_t = np.ascontiguousarray(w_s.transpose(1, 2, 0)).astype(NP_BF16)

    # Edge constants from the SHIPPED (bf16-rounded) weights and exact r:
    # mk[k, o] = sum_i W'b[i,k,o] * (-r_i); cL[w] = sum_{k<=3-w} mk, w=0..3;
    # cR[w'] = sum_{k>=5+w'... } — mirror: w=252..255 misses taps k>=260-w.
    w64 = w_t.astype(np.float64)  # [I,K,O]
    mk = -np.einsum("iko,i->ko", w64, r)  # [K,O]
    cL = np.stack([mk[0 : 4 - w].sum(axis=0) for w in range(4)], axis=1)  # [O,4]
    cR = np.stack([mk[8 - j :].sum(axis=0) for j in range(4)], axis=1)  # [O,4]
    rneg = (-r).astype(np.float64).reshape(C, 1)
    pr = np.concatenate([rneg, -rneg, cL, cR], axis=1).astype(np.float32)
    return w_t, np.ascontiguousarray(pr), r


def kernel(inputs, conv_w, gamma, beta, run_mean, run_var):
    global LAST_RESULTS
    conv_w, gamma, beta, run_mean, run_var = (
        np.asarray(a) for a in (conv_w, gamma, beta, run_mean, run_var)
    )
    w_t, pr, r = _prep_params(conv_w, gamma, beta, run_mean, run_var)
    x = np.asarray(inputs, dtype=np.float32)  # [B,C,H,W]
    rf = r.astype(np.float32)
    # h=0 state interior in bf16: x[:, :, 0] - r. x carries (h>=1) in bf16.
    n0 = (x[:, :, 0] - rf[None, :, None]).astype(NP_BF16)
    xq = np.ascontiguousarray(x[:, :, 1:, :]).astype(NP_BF16)
    in_maps = [
        dict(
            x=xq[c * BPC : (c + 1) * BPC],
            n0=n0[c * BPC : (c + 1) * BPC],
            w=w_t,
            pr=pr,
        )
        for c in range(NCORES)
    ]
    nc = _get_nc()
    trace = os.environ.get("KERNEL_TRACE", "0") == "1"
    res = run_bass_kernel_spmd(
        nc, in_maps, core_ids=list(range(NCORES)), trace=trace
    )
    LAST_RESULTS = res
    out = np.concatenate(
        [np.asarray(res.results[c]["o"]) for c in range(NCORES)], axis=0
    )[:, :, :, PAD : PAD + W].astype(np.float32)
    return out + rf[None, :, None, None]  # back to out-space


# revision 11
# speedup vs baseline: 1.0014x; 1.0014x over previous
"""Trainium2 Bass kernel: bidirectional conv-BN-relu message passing over H.

Reference semantics (per batch item, channels C, scan over H):
  forward:  new[0] = x[0];   new[h] = relu(bn(conv(new[h-1]))) + x[h]
  backward: out[H-1] = new[H-1]; out[h] = relu(bn(conv(out[h+1]))) + new[h]
conv = 1D conv along W, kernel 9, pad 4, C->C channels; BN (eval mode)
is a per-channel affine y*s + t.

Strategy: data-parallel over B across 8 cores (2 batch items per core).
Each conv step = 9 shifted-window matmuls accumulated in PSUM
(lhsT = per-tap [I,O] weights, rhs = state slice windows).

Matmul operands are bf16: fp32 weights disable the PE's fast-weight-load
path, while bf16 keeps the conv recurrence within ~7e-3 relative error
because PSUM accumulation and the x-carry adds stay fp32.

The affine+relu+carry tail is algebraically folded into ONE DVE op:
 - the BN scale s is folded into the weights host-side (W' = s[o]*W),
 - the state is stored shifted: n = new - r, where r solves
   r = t + M r (M[o,i] = sum_k W'[o,i,k]); then zero-padding of `new`
   corresponds to constant pads -r on n, and the update collapses to
     n_h = max(psum, -r) + carry          (single scalar_tensor_tensor)
   r is added back to the gathered output on the host.

Pad columns are NEVER streamed through the PE: each tap's matmul is
trimmed to the columns whose input window stays inside the valid
interior (N = 256-|k-4|), saving 20 of 2304 streamed columns per step.
The missing pad contributions are a precomputed [C,4]-per-edge constant
(host-solved from W' and r) added into PSUM by two small DVE ops before
the state-update STT — exact, not an approximation. This also deletes
all pad-fill maintenance of the state tiles.

The backward carry new_h = n_h + r is staged per step on the (otherwise
idle) Activation engine so the tensor engine runs nothing but the 9-tap
conv groups.

Startup/drain engineering (trace-driven): the engines cannot issue
anything before ~7.6us (framework preamble) and the first DMA packet
hits ~1.45us after issue, so the startup is bandwidth-bound on the
critical bytes. Those are minimized: x carries ship as bf16 (validated
+2e-4 error), h=0 state ships interior-only, and the three transfers
spread over three queues (sync: state+x, scalar: taps 0-3, gpsimd:
params+taps 4-8). A burst of dummy matmuls (on a vector-memset tile)
keeps the PE busy from ~7.6us so the HAM clock-gate lifts (1.2 ->
2.4 GHz) as early as possible. Input slices arrive in growing batches;
outputs leave pad-inclusive (contiguous 528B*OB runs) on two queues
mid-stream, and the final h=0 slices fan out across four queues so the
drain rides four DMA rings in parallel.
"""

import os
from contextlib import ExitStack

import numpy as np
import ml_dtypes

import bass_rust
import concourse.bass as bass
import concourse.tile as tile
from concourse import mybir
from concourse.bass_utils import run_bass_kernel_spmd

B, C, H, W = 16, 128, 64, 256
K, PAD = 9, 4
NCORES = 8
BPC = B // NCORES  # batch items per core
WP = W + 2 * PAD
EPS = 1e-5
OB = 4  # output-slice DMA batch
NWARM = 12  # dummy matmuls bridging the PE from preamble-end to the
# arrival of the first weight/state DMAs (~2.4us at the cold 213ns/mm)

F32 = mybir.dt.float32
BF16 = mybir.dt.bfloat16
NP_BF16 = ml_dtypes.bfloat16
IDENT = mybir.ActivationFunctionType.Identity

_NC_CACHE: dict = {}
LAST_RESULTS = None  # stashed BassKernelResults for test.py introspection


def _xbounds(h_dim):
    """Input-batch spans for h>=1: small leading batches so the first conv
    steps aren't gated on a bulk transfer, then steady groups of 4."""
    bounds, sizes, lo = [], [1, 2, 4], 1
    while lo < h_dim:
        sz = sizes[0] if sizes else 4
        if sizes:
            sizes = sizes[1:]
        bounds.append((lo, min(lo + sz, h_dim)))
        lo += sz
    return bounds


# Per-tap trimmed geometry: tap k covers psum cols [PO[k], PO[k]+NK[k])
# reading state interior cols [RO[k], RO[k]+NK[k]) of the padded row.
RO = [max(k, PAD) for k in range(K)]
PO = [max(0, PAD - k) for k in range(K)]
NK = [min(k + W, W + PAD) - max(k, PAD) for k in range(K)]


def _build_nc(bpc=BPC, h_dim=H, w_dim=W):
    wp = w_dim + 2 * PAD
    nc = bass.Bass()
    x_d = nc.dram_tensor("x", [bpc, C, h_dim - 1, w_dim], BF16, kind="ExternalInput")
    n0_d = nc.dram_tensor("n0", [bpc, C, w_dim], BF16, kind="ExternalInput")
    w_d = nc.dram_tensor("w", [C, K, C], BF16, kind="ExternalInput")
    # pr: [-r, +r, cL(4), cR(4)] per channel
    pr_d = nc.dram_tensor("pr", [C, 10], F32, kind="ExternalInput")
    o_d = nc.dram_tensor("o", [bpc, C, h_dim, wp], BF16, kind="ExternalOutput")

    add = mybir.AluOpType.add
    mx = mybir.AluOpType.max

    xb_list = _xbounds(h_dim)
    xb_of_h = {}
    for i, (lo, hi) in enumerate(xb_list):
        for h in range(lo, hi):
            xb_of_h[h] = (i, lo, hi)

    with ExitStack() as ctx:
        tc = ctx.enter_context(tile.TileContext(nc))
        singles = ctx.enter_context(tc.tile_pool(name="singles", bufs=1))
        big = ctx.enter_context(tc.tile_pool(name="big", bufs=1))
        xs_pool = ctx.enter_context(tc.tile_pool(name="xs", bufs=6))
        nr_pool = ctx.enter_context(tc.tile_pool(name="nr", bufs=8))
        pp = ctx.enter_context(tc.tile_pool(name="pp", bufs=8, space="PSUM"))

        # --- DMA ordering: three queues in parallel. sync: h=0 state
        # (gates the first conv group) then the x-carry batches; scalar:
        # weight taps 0-3 (gate the first matmuls); gpsimd: the tiny param
        # vector then taps 4-8 (needed ~1us after tap 0).
        new = []
        for c in range(bpc):
            nt = big.tile([C, h_dim, wp], BF16, tag=f"new{c}", name=f"new{c}")
            nc.sync.dma_start(out=nt[:, 0, PAD : PAD + w_dim], in_=n0_d[c])
            new.append(nt)

        prt = singles.tile([C, 10], F32, tag="prt", name="prt")
        nc.gpsimd.dma_start(out=prt, in_=pr_d[:, :])
        bt = prt[:, 0:1]
        rt = prt[:, 1:2]
        cl = prt[:, 2:6]
        cr = prt[:, 6:10]

        wt = singles.tile([C, K, C], BF16, tag="wt", name="wt")
        nc.scalar.dma_start(out=wt[:, 0:4, :], in_=w_d[:, 0:4, :])
        nc.gpsimd.dma_start(out=wt[:, 4:K, :], in_=w_d[:, 4:K, :])
        wr = [wt[:, k, :] for k in range(K)]

        # --- HAM warmup: dummy matmuls on a zeroed tile keep the PE busy
        # while the DMAs land, releasing the activity clock-gate.
        dummy = singles.tile([C, w_dim], BF16, tag="dummy", name="dummy")
        nc.vector.memset(dummy, 0.0)
        wm = pp.tile([C, w_dim], F32, tag="pt", name="wm", bufs=6)
        for _ in range(NWARM):
            nc.tensor.matmul(wm, dummy[:, 0:C], dummy, start=True, stop=True)

        # Both 4-wide edge windows of a psum tile (and of the [C,8] constant)
        # as one 3D AP, so the pad fixup is a single DVE op — the DVE chain
        # per group (fix + STT) must stay comfortably under the ~975ns PE
        # group time or every group boundary stalls on the state-row RAW.
        def edges2(ap4, gap):
            return bass.AP(ap4.tensor, ap4.offset, [ap4.ap[0], [gap, 2], [1, PAD]])

        clr = edges2(cl, PAD)  # prt cols 2:6 + 6:10

        def conv_group(src_row, pt):
            """9 edge-trimmed taps accumulated in PSUM + pad-constant fixup."""
            for k in range(K):
                nc.tensor.matmul(
                    pt[:, PO[k] : PO[k] + NK[k]],
                    wr[k],
                    src_row[:, RO[k] : RO[k] + NK[k]],
                    start=(k == 0),
                    stop=(k == K - 1),
                )
            pe = edges2(pt[:, 0:PAD], w_dim - PAD)
            nc.vector.tensor_add(out=pe, in0=pe, in1=clr)

        # Forward scan over H (both chains interleaved per h).
        xtiles: list[dict[int, object]] = [dict() for _ in range(bpc)]
        for h in range(1, h_dim):
            bi, lo, hi = xb_of_h[h]
            if h == lo:
                for c in range(bpc):
                    xb = xs_pool.tile([C, 4, w_dim], BF16, tag="xb", name="xb")
                    nc.sync.dma_start(
                        out=xb[:, 0 : hi - lo, :], in_=x_d[c][:, lo - 1 : hi - 1, :]
                    )
                    xtiles[c][bi] = xb
            for c in range(bpc):
                pt = pp.tile([C, w_dim], F32, tag="pt", name="pt", bufs=6)
                conv_group(new[c][:, h - 1, :], pt)
                nc.vector.scalar_tensor_tensor(
                    out=new[c][:, h, PAD : PAD + w_dim],
                    in0=pt,
                    scalar=bt,
                    in1=xtiles[c][bi][:, h - lo, :],
                    op0=mx,
                    op1=add,
                )

        # Backward scan; out[h] overwrites new[h] in place, then streams out
        # pad-inclusive in batches of OB slices (contiguous runs DMA much
        # faster than pad-strided 512B packets), one queue per chain.
        oq = [nc.scalar, nc.sync]
        for h in range(h_dim - 2, 0, -1):
            for c in range(bpc):
                # Stage the true backward carry new_h = n_h + r on the ACT
                # engine (reads the forward state before it's overwritten).
                nr = nr_pool.tile([C, w_dim], F32, tag="nr", name="nr")
                nc.scalar.activation(
                    out=nr, in_=new[c][:, h, PAD : PAD + w_dim],
                    func=IDENT, bias=rt, scale=1.0,
                )
                pt = pp.tile([C, w_dim], F32, tag="pt", name="pt", bufs=6)
                conv_group(new[c][:, h + 1, :], pt)
                nc.vector.scalar_tensor_tensor(
                    out=new[c][:, h, PAD : PAD + w_dim],
                    in0=pt,
                    scalar=bt,
                    in1=nr,
                    op0=mx,
                    op1=add,
                )
            if h == 2:
                # Split the final OB-batch so the very last transfer (which
                # gates the drain) is only 2 slices.
                for c in range(bpc):
                    oq[c % 2].dma_start(
                        out=o_d[c][:, 2:4, :], in_=new[c][:, 2:4, :]
                    )
            elif h == 1:
                for c in range(bpc):
                    oq[c % 2].dma_start(
                        out=o_d[c][:, 1:2, :], in_=new[c][:, 1:2, :]
                    )
            elif h % OB == 0:
                hi = min(h + OB, h_dim)
                for c in range(bpc):
                    oq[c % 2].dma_start(
                        out=o_d[c][:, h:hi, :], in_=new[c][:, h:hi, :]
                    )

        # Final step (h=0) in two half-width PSUM groups per chain so the
        # very last DVE op and output transfer are half-sized — they sit on
        # the kernel's drain critical path. Each of the four half-slices
        # leaves on its own DMA queue so the drain transfers run on four
        # rings in parallel.
        hw2 = w_dim // 2
        # Only sync/scalar/gpsimd can issue DMAs. The two half1 pieces are
        # the trailing ones — keep them on different queues.
        oq0 = [[nc.scalar, nc.gpsimd], [nc.sync, nc.scalar]]
        for c in range(bpc):
            nr = nr_pool.tile([C, w_dim], F32, tag="nr", name="nr")
            nc.scalar.activation(
                out=nr, in_=new[c][:, 0, PAD : PAD + w_dim],
                func=IDENT, bias=rt, scale=1.0,
            )
            for half in range(2):
                lo = half * hw2
                pt = pp.tile([C, hw2], F32, tag="pth", name="pth", bufs=2)
                for k in range(K):
                    # Trim within this half: the left edge only exists in
                    # half 0, the right edge only in half 1.
                    if half == 0:
                        p0, p1 = PO[k], hw2
                        r0 = RO[k]
                    else:
                        p0, p1 = 0, min(hw2, hw2 + PAD - k)
                        r0 = k + hw2
                    nc.tensor.matmul(
                        pt[:, p0:p1],
                        wr[k],
                        new[c][:, 1, r0 : r0 + (p1 - p0)],
                        start=(k == 0),
                        stop=(k == K - 1),
                    )
                if half == 0:
                    nc.vector.tensor_add(
                        out=pt[:, 0:PAD], in0=pt[:, 0:PAD], in1=cl
                    )
                else:
                    nc.vector.tensor_add(
                        out=pt[:, hw2 - PAD : hw2],
                        in0=pt[:, hw2 - PAD : hw2],
                        in1=cr,
                    )
                nc.vector.scalar_tensor_tensor(
                    out=new[c][:, 0, PAD + lo : PAD + lo + hw2],
                    in0=pt,
                    scalar=bt,
                    in1=nr[:, lo : lo + hw2],
                    op0=mx,
                    op1=add,
                )
                # Pad-inclusive half-slice: left half carries the left pads,
                # right half the right pads.
                if half == 0:
                    oq0[c][half].dma_start(
                        out=o_d[c][:, 0, 0 : PAD + hw2],
                        in_=new[c][:, 0, 0 : PAD + hw2],
                    )
                else:
                    oq0[c][half].dma_start(
                        out=o_d[c][:, 0, PAD + hw2 : wp],
                        in_=new[c][:, 0, PAD + hw2 : wp],
                    )

    # TRN2 caps most instructions at one semaphore wait (matmuls lower to an
    # LDWEIGHTS struct with a single wait slot); split any excess onto
    # EventSemaphore instructions like bacc does.
    bass_rust.generate_event_semaphores(nc)
    return nc


def _get_nc():
    key = (BPC, H, W)
    if key not in _NC_CACHE:
        _NC_CACHE[key] = _build_nc()
    return _NC_CACHE[key]


def _prep_params(conv_w, gamma, beta, run_mean, run_var):
    """Fold BN scale into the weights, solve the state shift r, and build
    the pad-contribution edge constants.

    Returns (w_t [I,K,O] bf16 with s folded, pr [C,10] f32, r [C] f64).
    """
    s = gamma.astype(np.float64) / np.sqrt(run_var.astype(np.float64) + EPS)
    t = beta.astype(np.float64) - run_mean.astype(np.float64) * s
    w_s = s[:, None, None] * conv_w.astype(np.float64)  # [O,I,K]
    m = w_s.sum(axis=2)  # [O,I]
    r = np.linalg.solve(np.eye(C) - m, t)
    w_t = np.ascontiguousarray(w_s.transpose(1, 2, 0)).astype(NP_BF16)

    # Edge constants from the SHIPPED (bf16-rounded) weights and exact r:
    # mk[k, o] = sum_i W'b[i,k,o] * (-r_i); cL[w] = sum_{k<=3-w} mk, w=0..3;
    # cR[w'] = sum_{k>=5+w'... } — mirror: w=252..255 misses taps k>=260-w.
    w64 = w_t.astype(np.float64)  # [I,K,O]
    mk = -np.einsum("iko,i->ko", w64, r)  # [K,O]
    cL = np.stack([mk[0 : 4 - w].sum(axis=0) for w in range(4)], axis=1)  # [O,4]
    cR = np.stack([mk[8 - j :].sum(axis=0) for j in range(4)], axis=1)  # [O,4]
    rneg = (-r).astype(np.float64).reshape(C, 1)
    pr = np.concatenate([rneg, -rneg, cL, cR], axis=1).astype(np.float32)
    return w_t, np.ascontiguousarray(pr), r


def kernel(inputs, conv_w, gamma, beta, run_mean, run_var):
    global LAST_RESULTS
    conv_w, gamma, beta, run_mean, run_var = (
        np.asarray(a) for a in (conv_w, gamma, beta, run_mean, run_var)
    )
    w_t, pr, r = _prep_params(conv_w, gamma, beta, run_mean, run_var)
    x = np.asarray(inputs, dtype=np.float32)  # [B,C,H,W]
    rf = r.astype(np.float32)
    # h=0 state interior in bf16: x[:, :, 0] - r. x carries (h>=1) in bf16.
    n0 = (x[:, :, 0] - rf[None, :, None]).astype(NP_BF16)
    xq = np.ascontiguousarray(x[:, :, 1:, :]).astype(NP_BF16)
    in_maps = [
        dict(
            x=xq[c * BPC : (c + 1) * BPC],
            n0=n0[c * BPC : (c + 1) * BPC],
            w=w_t,
            pr=pr,
        )
        for c in range(NCORES)
    ]
    nc = _get_nc()
    trace = os.environ.get("KERNEL_TRACE", "0") == "1"
    res = run_bass_kernel_spmd(
        nc, in_maps, core_ids=list(range(NCORES)), trace=trace
    )
    LAST_RESULTS = res
    out = np.concatenate(
        [np.asarray(res.results[c]["o"]) for c in range(NCORES)], axis=0
    )[:, :, :, PAD : PAD + W].astype(np.float32)
    return out + rf[None, :, None, None]  # back to out-space


# revision 12
# speedup vs baseline: 1.0329x; 1.0315x over previous
"""Trainium2 Bass kernel: bidirectional conv-BN-relu message passing over H.

Reference semantics (per batch item, channels C, scan over H):
  forward:  new[0] = x[0];   new[h] = relu(bn(conv(new[h-1]))) + x[h]
  backward: out[H-1] = new[H-1]; out[h] = relu(bn(conv(out[h+1]))) + new[h]
conv = 1D conv along W, kernel 9, pad 4, C->C channels; BN (eval mode)
is a per-channel affine y*s + t.

Strategy: data-parallel over B across 8 cores (2 batch items per core).
Each conv step = 9 shifted-window matmuls accumulated in PSUM
(lhsT = per-tap [I,O] weights, rhs = padded state slice windows).
Matmul operands are bf16 (fp32 weights would disable fast-weight-load);
PSUM accumulation and the carry adds stay fp32 (~7e-3 relative error).

State-variable choice (the key scheduling trick): the BN scale s is
folded into the weights host-side (W' = s[o]*W), and the affine shift t
is folded into the STATES so each step's affine+relu+carry collapses to
ONE DVE scalar_tensor_tensor and the backward needs NO carry-prep op:
 - forward state  w(h) = new(h) + t, pad columns = t. Then
   conv(w-padded) = conv(new) + M t (M[o,i] = sum_k W'[o,i,k]) and
     w(h) = max(psum, q) + [x(h) + 2t - Mt],   q = Mt - t
   with the bracket baked into the shipped x slices (bf16, validated).
 - backward state v(h) = out(h) (true out-space), pad columns = 0,
   written to a SEPARATE tile (outT):
     v(h) = max(psum_v, -t) + w(h)
   The carry is the stored forward row itself — no ACT op, and writing
   v to its own tile keeps the backward reads of w free of WAR hazards
   against recent writes (tile dep tracking is coarse per-tile).
 - the forward h=63 step writes v(63) = w(63) - t directly into outT by
   shipping that one x slice with offset (t - Mt) instead of (2t - Mt).
Outputs leave in out-space: the host just slices off the pads.

Startup/drain engineering (trace-driven): engines cannot issue anything
before ~7.6us (framework preamble) and the first DMA packet lands
~1.45us after issue, so the startup is bandwidth-bound on the critical
bytes — x carries ship as bf16 (validated +2e-4 error), h=0 state ships
interior-only, weights lead the scalar queue in two chunks while state
and x ride sync. A burst of dummy matmuls (on a vector-memset tile)
holds the PE busy so the HAM clock-gate lifts (1.2 -> 2.4 GHz) before
the real stream. ONLY the sync and scalar DMA rings are used: the
gpsimd ring is ~5x slower and its exit DRAIN then gates the teardown
barrier. Outputs leave pad-inclusive (contiguous 528B*OB runs) on both
queues mid-stream; the final h=0 slices go out as four half-width
pieces, two per ring, so the drain transfers overlap.
"""

import os
from contextlib import ExitStack

import numpy as np
import ml_dtypes

import bass_rust
import concourse.bass as bass
import concourse.tile as tile
from concourse import mybir
from concourse.bass_utils import run_bass_kernel_spmd

B, C, H, W = 16, 128, 64, 256
K, PAD = 9, 4
NCORES = 8
BPC = B // NCORES  # batch items per core
WP = W + 2 * PAD
EPS = 1e-5
OB = 4  # output-slice DMA batch
NWARM = 12  # dummy matmuls bridging the PE from preamble-end to the
# arrival of the first weight/state DMAs (~2.5us at the cold 213ns/mm)

F32 = mybir.dt.float32
BF16 = mybir.dt.bfloat16
NP_BF16 = ml_dtypes.bfloat16

_NC_CACHE: dict = {}
LAST_RESULTS = None  # stashed BassKernelResults for test.py introspection


def _xbounds(h_dim):
    """Input-batch spans for h>=1: small leading batches so the first conv
    steps aren't gated on a bulk transfer, then steady groups of 4."""
    bounds, sizes, lo = [], [1, 2, 4], 1
    while lo < h_dim:
        sz = sizes[0] if sizes else 4
        if sizes:
            sizes = sizes[1:]
        bounds.append((lo, min(lo + sz, h_dim)))
        lo += sz
    return bounds


def _build_nc(bpc=BPC, h_dim=H, w_dim=W):
    wp = w_dim + 2 * PAD
    nc = bass.Bass()
    x_d = nc.dram_tensor("x", [bpc, C, h_dim - 1, w_dim], BF16, kind="ExternalInput")
    n0_d = nc.dram_tensor("n0", [bpc, C, w_dim], BF16, kind="ExternalInput")
    w_d = nc.dram_tensor("w", [C, K, C], BF16, kind="ExternalInput")
    # pr: [q = Mt - t, -t, t] per channel
    pr_d = nc.dram_tensor("pr", [C, 3], F32, kind="ExternalInput")
    o_d = nc.dram_tensor("o", [bpc, C, h_dim, wp], BF16, kind="ExternalOutput")

    add = mybir.AluOpType.add
    mx = mybir.AluOpType.max

    xb_list = _xbounds(h_dim)
    xb_of_h = {}
    for i, (lo, hi) in enumerate(xb_list):
        for h in range(lo, hi):
            xb_of_h[h] = (i, lo, hi)

    with ExitStack() as ctx:
        tc = ctx.enter_context(tile.TileContext(nc))
        singles = ctx.enter_context(tc.tile_pool(name="singles", bufs=1))
        big = ctx.enter_context(tc.tile_pool(name="big", bufs=1))
        xs_pool = ctx.enter_context(tc.tile_pool(name="xs", bufs=6))
        pp = ctx.enter_context(tc.tile_pool(name="pp", bufs=8, space="PSUM"))

        # --- DMA ordering (sync + scalar rings only): sync: tiny params,
        # h=0 state (gates the first conv group), then the x batches;
        # scalar: weight taps 0-4 then 5-8 (tap 0 gates the first matmul).
        prt = singles.tile([C, 3], F32, tag="prt", name="prt")
        nc.sync.dma_start(out=prt, in_=pr_d[:, :])
        qt = prt[:, 0:1]
        nt = prt[:, 1:2]
        tt = prt[:, 2:3]

        new, outT = [], []
        for c in range(bpc):
            wtile = big.tile([C, h_dim, wp], BF16, tag=f"new{c}", name=f"new{c}")
            nc.sync.dma_start(out=wtile[:, 0, PAD : PAD + w_dim], in_=n0_d[c])
            new.append(wtile)
            ot = big.tile([C, h_dim, wp], BF16, tag=f"out{c}", name=f"out{c}")
            outT.append(ot)

        wt = singles.tile([C, K, C], BF16, tag="wt", name="wt")
        nc.scalar.dma_start(out=wt[:, 0:5, :], in_=w_d[:, 0:5, :])
        nc.scalar.dma_start(out=wt[:, 5:K, :], in_=w_d[:, 5:K, :])
        wr = [wt[:, k, :] for k in range(K)]

        # --- HAM warmup: dummy matmuls on a zeroed tile keep the PE busy
        # while the DMAs land, releasing the activity clock-gate.
        dummy = singles.tile([C, w_dim], BF16, tag="dummy", name="dummy")
        nc.vector.memset(dummy, 0.0)
        wm = pp.tile([C, w_dim], F32, tag="pt", name="wm", bufs=4)
        for _ in range(NWARM):
            nc.tensor.matmul(wm, dummy[:, 0:C], dummy, start=True, stop=True)

        # Pad columns: w rows 0..62 hold t; v rows 0..63 hold 0. (w row 63
        # is never written or read — the h=63 result goes to outT.)
        zp = singles.tile([C, h_dim - 1, 2 * PAD], F32, tag="zp", name="zp")
        nc.vector.memset(zp, 0.0)
        for c in range(bpc):
            nc.vector.tensor_scalar(
                out=new[c][:, 0 : h_dim - 1, 0:PAD], in0=zp[:, :, 0:PAD],
                scalar1=tt, scalar2=None, op0=add,
            )
            nc.vector.tensor_scalar(
                out=new[c][:, 0 : h_dim - 1, PAD + w_dim : wp],
                in0=zp[:, :, PAD : 2 * PAD],
                scalar1=tt, scalar2=None, op0=add,
            )
            nc.vector.memset(outT[c][:, :, 0:PAD], 0.0)
            nc.vector.memset(outT[c][:, :, PAD + w_dim : wp], 0.0)

        def conv_group(src_row, pt):
            for k in range(K):
                nc.tensor.matmul(
                    pt,
                    wr[k],
                    src_row[:, k : k + w_dim],
                    start=(k == 0),
                    stop=(k == K - 1),
                )

        # Forward scan over H (both chains interleaved per h). h=63 writes
        # out-space directly into outT (its x slice ships with offset t-Mt).
        xtiles: list[dict[int, object]] = [dict() for _ in range(bpc)]
        for h in range(1, h_dim):
            bi, lo, hi = xb_of_h[h]
            if h == lo:
                for c in range(bpc):
                    xb = xs_pool.tile([C, 4, w_dim], BF16, tag="xb", name="xb")
                    nc.sync.dma_start(
                        out=xb[:, 0 : hi - lo, :], in_=x_d[c][:, lo - 1 : hi - 1, :]
                    )
                    xtiles[c][bi] = xb
            for c in range(bpc):
                pt = pp.tile([C, w_dim], F32, tag="pt", name="pt", bufs=4)
                conv_group(new[c][:, h - 1, :], pt)
                dst = outT[c] if h == h_dim - 1 else new[c]
                nc.vector.scalar_tensor_tensor(
                    out=dst[:, h, PAD : PAD + w_dim],
                    in0=pt,
                    scalar=qt,
                    in1=xtiles[c][bi][:, h - lo, :],
                    op0=mx,
                    op1=add,
                )

        # Backward scan: v(h) = max(conv(v(h+1)), -t) + w(h), written to
        # outT; slices stream out pad-inclusive in OB batches (contiguous
        # 528B*OB runs DMA much faster than pad-strided 512B packets).
        oq = [nc.scalar, nc.sync]
        for h in range(h_dim - 2, 0, -1):
            for c in range(bpc):
                pt = pp.tile([C, w_dim], F32, tag="pt", name="pt", bufs=4)
                conv_group(outT[c][:, h + 1, :], pt)
                nc.vector.scalar_tensor_tensor(
                    out=outT[c][:, h, PAD : PAD + w_dim],
                    in0=pt,
                    scalar=nt,
                    in1=new[c][:, h, PAD : PAD + w_dim],
                    op0=mx,
                    op1=add,
                )
            if h == 2:
                # Split the final OB-batch so the very last transfers (which
                # gate the drain) are small.
                for c in range(bpc):
                    oq[c % 2].dma_start(
                        out=o_d[c][:, 2:4, :], in_=outT[c][:, 2:4, :]
                    )
            elif h == 1:
                for c in range(bpc):
                    oq[c % 2].dma_start(
                        out=o_d[c][:, 1:2, :], in_=outT[c][:, 1:2, :]
                    )
            elif h % OB == 0:
                hi = min(h + OB, h_dim)
                for c in range(bpc):
                    oq[c % 2].dma_start(
                        out=o_d[c][:, h:hi, :], in_=outT[c][:, h:hi, :]
                    )

        # Final step (h=0) in two half-width PSUM groups per chain so the
        # very last DVE op and output transfers are half-sized — they sit
        # on the kernel's drain critical path. Four pieces, two per ring.
        hw2 = w_dim // 2
        for c in range(bpc):
            for half in range(2):
                lo = half * hw2
                pt = pp.tile([C, hw2], F32, tag="pth", name="pth", bufs=2)
                for k in range(K):
                    nc.tensor.matmul(
                        pt,
                        wr[k],
                        outT[c][:, 1, lo + k : lo + k + hw2],
                        start=(k == 0),
                        stop=(k == K - 1),
                    )
                nc.vector.scalar_tensor_tensor(
                    out=outT[c][:, 0, PAD + lo : PAD + lo + hw2],
                    in0=pt,
                    scalar=nt,
                    in1=new[c][:, 0, PAD + lo : PAD + lo + hw2],
                    op0=mx,
                    op1=add,
                )
                # Pad-inclusive half-slice: left half carries the left pads,
                # right half the right pads.
                if half == 0:
                    oq[c % 2].dma_start(
                        out=o_d[c][:, 0, 0 : PAD + hw2],
                        in_=outT[c][:, 0, 0 : PAD + hw2],
                    )
                else:
                    oq[(c + 1) % 2].dma_start(
                        out=o_d[c][:, 0, PAD + hw2 : wp],
                        in_=outT[c][:, 0, PAD + hw2 : wp],
                    )

    # TRN2 caps most instructions at one semaphore wait (matmuls lower to an
    # LDWEIGHTS struct with a single wait slot); split any excess onto
    # EventSemaphore instructions like bacc does.
    bass_rust.generate_event_semaphores(nc)
    return nc


def _get_nc():
    key = (BPC, H, W)
    if key not in _NC_CACHE:
        _NC_CACHE[key] = _build_nc()
    return _NC_CACHE[key]


def _prep_params(conv_w, gamma, beta, run_mean, run_var):
    """Fold BN scale into the weights; build the t-shift constants.

    Returns (w_t [I,K,O] bf16 with s folded, pr [C,3] f32, t, Mt [C] f64).
    """
    s = gamma.astype(np.float64) / np.sqrt(run_var.astype(np.float64) + EPS)
    t = beta.astype(np.float64) - run_mean.astype(np.float64) * s
    w_s = s[:, None, None] * conv_w.astype(np.float64)  # [O,I,K]
    w_t = np.ascontiguousarray(w_s.transpose(1, 2, 0)).astype(NP_BF16)

    # Mt from the SHIPPED (bf16-rounded) weights and the bf16-rounded t
    # that actually sits in the pad columns, so the algebra matches the
    # device bit-for-bit up to fp32 accumulation.
    w64 = w_t.astype(np.float64)  # [I,K,O]
    tb = t.astype(np.float32).astype(NP_BF16).astype(np.float64)
    mt = np.einsum("iko,i->o", w64, tb)  # [O]
    q = mt - t
    pr = np.stack([q, -t, t], axis=1).astype(np.float32)
    return w_t, np.ascontiguousarray(pr), t, mt


def kernel(inputs, conv_w, gamma, beta, run_mean, run_var):
    global LAST_RESULTS
    conv_w, gamma, beta, run_mean, run_var = (
        np.asarray(a) for a in (conv_w, gamma, beta, run_mean, run_var)
    )
    w_t, pr, t, mt = _prep_params(conv_w, gamma, beta, run_mean, run_var)
    x = np.asarray(inputs, dtype=np.float64)  # [B,C,H,W]
    # h=0 state interior: w(0) = x[:,:,0] + t. x carries for h=1..62 ship
    # with offset (2t - Mt); the h=63 slice with (t - Mt) so that step
    # lands directly in out-space.
    n0 = (x[:, :, 0] + t[None, :, None]).astype(NP_BF16)
    off = (2.0 * t - mt)[None, :, None, None]
    xq = x[:, :, 1:, :] + off
    xq[:, :, -1, :] -= t[None, :, None]
    xq = np.ascontiguousarray(xq).astype(NP_BF16)
    in_maps = [
        dict(
            x=xq[c * BPC : (c + 1) * BPC],
            n0=n0[c * BPC : (c + 1) * BPC],
            w=w_t,
            pr=pr,
        )
        for c in range(NCORES)
    ]
    nc = _get_nc()
    trace = os.environ.get("KERNEL_TRACE", "0") == "1"
    res = run_bass_kernel_spmd(
        nc, in_maps, core_ids=list(range(NCORES)), trace=trace
    )
    LAST_RESULTS = res
    out = np.concatenate(
        [np.asarray(res.results[c]["o"]) for c in range(NCORES)], axis=0
    )[:, :, :, PAD : PAD + W].astype(np.float32)
    return out  # already in out-space
